# revision 3
# baseline (speedup 1.0000x reference)
"""L2 (spectral) contrastive loss on 8 Trainium2 NeuronCores.

Math: with G_x = x.T @ x and G_y = y.T @ y (both [D, D]),
    sum_{i,j} <x_i, y_j>^2 = ||x @ y.T||_F^2 = tr(G_x @ G_y) = sum(G_x * G_y)
so the loss needs only the two Gram matrices (2*N*D^2 MACs) instead of the
[N, N] pairwise product (N^2*D MACs).

v2 split-collective structure (vs the single 1.38MB AllReduce baseline):
  - Rows are split across the 8 cores. Only G_y's partials cross the wire:
      sum(Gx_tot * Gy_tot) = sum_c sum(Gx_c * Gy_tot)
    so AllReduce #1 carries just the G_y triangle (fp16, 688KB), each core
    dots its LOCAL Gx partial against the summed Gy, and AllReduce #2
    carries only [1, 2] f32 scalars.
  - Inputs stream on 4 DMA queues (y chunks first), fp8 casts on the
    scalar engine (gpsimd casts were 6x slower and the old critical path),
    G_y computed kk-outer with all 6 triangle slabs resident in PSUM
    (exactly 8 banks) so the Gram finishes with the last y cast and the
    first collective triggers ~30µs earlier than baseline.
  - z_i = <x_i, y_i> on the vector engine from fp32; z terms ride
    AllReduce #2 in exact f32.
  (A 3-round remote-DMA butterfly is faster than the collective but
  intermittently wedges the device - keep the firmware collectives.)
"""
import numpy as np
from contextlib import ExitStack

from concourse import bacc, tile, mybir
from concourse.bass_utils import run_bass_kernel_spmd

N_CORES = 8
N, D = 8192, 768
ROWS = N // N_CORES          # 1024 rows per core
P = 128                      # SBUF partitions
KCH = ROWS // P              # 8 contraction chunks per core
KK = KCH // 2                # 4 DoubleRow steps (2 chunks per pass)
MS = D // P                  # 6 output slabs per Gram

WIDTHS = [D - P * m for m in range(MS)]              # [768,640,512,384,256,128]
COFF = [sum(WIDTHS[:m]) for m in range(MS)]          # prefix offsets
GCOLS = sum(WIDTHS)                                  # 2688 triangle cols

F32 = mybir.dt.float32
F16 = mybir.dt.float16
FP8 = mybir.dt.float8e4

# Gx pre-scale: keeps fp16 dot products (diag ~8192 * 8192) in fp16 range
SCALE = 2.0 ** -13

_CACHE = {}


def _mm_chunks(width):
    """Split [0, width) at the 512-column PSUM bank boundary."""
    if width <= 512:
        return [(0, width)]
    return [(0, 512), (512, width)]


def _build():
    nc = bacc.Bacc("TRN2", target_bir_lowering=False, debug=False,
                   num_devices=N_CORES)
    x_ap = nc.dram_tensor("x", [ROWS, D], F32, kind="ExternalInput").ap()
    y_ap = nc.dram_tensor("y", [ROWS, D], F32, kind="ExternalInput").ap()
    loss_ap = nc.dram_tensor("loss", [1, 1], F32, kind="ExternalOutput").ap()

    inv_nn1 = 1.0 / (float(N) * (N - 1))

    with tile.TileContext(nc) as tc:
        with ExitStack() as ctx:
            sb = ctx.enter_context(tc.tile_pool(name="sb", bufs=1))
            ps = ctx.enter_context(tc.tile_pool(name="ps", bufs=1, space="PSUM"))
            dram = ctx.enter_context(tc.tile_pool(name="dram", bufs=1,
                                                  space="DRAM"))

            # ---- load inputs: [1024, 768] -> [128p, 8k, 768] on 4 DMA
            # queues, ALL y chunks issued before any x so G_y (the only
            # Gram that rides the big collective) completes first ----
            xt = sb.tile([P, KCH, D], F32)
            yt = sb.tile([P, KCH, D], F32)
            xr = x_ap.rearrange("(n p) d -> p n d", p=P)
            yr = y_ap.rearrange("(n p) d -> p n d", p=P)
            qeng = (nc.sync, nc.scalar, nc.gpsimd)
            for k in range(KCH):
                qeng[k % 3].dma_start(yt[:, k, :], yr[:, k, :])
            for k in range(KCH):
                qeng[k % 3].dma_start(xt[:, k, :], xr[:, k, :])

            # ---- fp8 casts, all on the scalar engine (~0.7µs each; the
            # old gpsimd path was 3.2µs each and the critical path) ----
            yb = sb.tile([P, KCH, D], FP8)
            xb = sb.tile([P, KCH, D], FP8)
            for k in range(KCH):
                nc.scalar.copy(yb[:, k, :], yt[:, k, :])
            for k in range(KCH):
                nc.scalar.copy(xb[:, k, :], xt[:, k, :])

            ones = sb.tile([P, 1], F32)
            nc.vector.memset(ones[:], 1.0)

            # ---- G_y: kk-outer with all 6 triangle slabs resident in
            # PSUM (2+2+1+1+1+1 = exactly 8 banks), consuming each cast
            # chunk pair as it arrives ----
            slabs = []
            for m in range(MS):
                slab = ps.tile([P, WIDTHS[m]], F32, tag=f"slab{m}", bufs=1,
                               name=f"slab{m}")
                slabs.append(slab)

            pack = sb.tile([P, GCOLS], F16)       # G_y fp16 AR payload
            gxp = sb.tile([P, GCOLS], F16)        # local Gx, scaled, x2 off-diag

            def gram_pass(src):
                for kk in range(KK):
                    for m in range(MS):
                        for (c0, c1) in _mm_chunks(WIDTHS[m]):
                            nc.tensor.matmul(
                                slabs[m][:, c0:c1],
                                src[:, 2 * kk:2 * kk + 2, P * m:P * (m + 1)],
                                src[:, 2 * kk:2 * kk + 2,
                                    P * m + c0:P * m + c1],
                                start=(kk == 0),
                                stop=(kk == KK - 1),
                                perf_mode=mybir.MatmulPerfMode.DoubleRow,
                                skip_group_check=True,
                            )

            gram_pass(yb)

            # ---- pack G_y slabs to fp16 (vector) and stage to DRAM in 3
            # pieces (gpsimd queue) so the collective triggers as soon as
            # the last slab lands ----
            cin = dram.tile([P, GCOLS], F16)
            cout = dram.tile([P, GCOLS], F16, addr_space="Shared")
            for m in range(MS):
                off, w = COFF[m], WIDTHS[m]
                nc.vector.tensor_copy(pack[:, off:off + w], slabs[m][:, 0:w])
                if m % 2 == 1:  # stage after slabs {0,1}, {2,3}, {4,5} pack
                    a = COFF[m - 1]
                    b = off + w
                    nc.gpsimd.dma_start(cin[:, a:b], pack[:, a:b])

            nc.gpsimd.collective_compute(
                "AllReduce",
                mybir.AluOpType.add,
                replica_groups=[list(range(N_CORES))],
                ins=[cin.opt()],
                outs=[cout.opt()],
            )

            # ---- G_x into the same PSUM slabs (WAR on the pack copies);
            # packed locally with the 2^-13 pre-scale and the x2 weight on
            # strict-right (off-diagonal) columns; never leaves the core ----
            gram_pass(xb)
            for m in range(MS):
                off, w = COFF[m], WIDTHS[m]
                nc.scalar.mul(gxp[:, off:off + P], slabs[m][:, 0:P], SCALE)
                if w > P:
                    nc.scalar.mul(gxp[:, off + P:off + w],
                                  slabs[m][:, P:w], 2.0 * SCALE)

            # ---- diagonal terms z_i = <x_i, y_i> from fp32 (vector) ----
            zscr = sb.tile([P, D], F32)
            zcols = sb.tile([P, KCH], F32)
            for k in range(KCH):
                nc.vector.scalar_tensor_tensor(
                    zscr[:], xt[:, k, :], 1.0, yt[:, k, :],
                    mybir.AluOpType.mult, mybir.AluOpType.mult,
                    accum_out=zcols[:, k:k + 1],
                )
            zsq = sb.tile([P, KCH], F32)
            stage = sb.tile([P, 5], F32)
            # stage col 3 = (2/N)*sum_k z, col 4 = inv_nn1*sum_k z^2
            zred = sb.tile([P, 2], F32)
            nc.vector.tensor_reduce(zred[:, 0:1], zcols[:],
                                    mybir.AxisListType.X, mybir.AluOpType.add)
            nc.vector.scalar_tensor_tensor(
                zsq[:], zcols[:], 1.0, zcols[:],
                mybir.AluOpType.mult, mybir.AluOpType.mult,
                accum_out=zred[:, 1:2],
            )
            nc.vector.tensor_scalar_mul(stage[:, 3:4], zred[:, 0:1], 2.0 / N)
            nc.vector.tensor_scalar_mul(stage[:, 4:5], zred[:, 1:2], inv_nn1)

            # ---- after AR1: read back Gy_tot in 3 pieces on 3 queues,
            # dot against the local Gx pack (3 fp16 STTs, f32 col accums) ----
            TH = GCOLS // 3
            gsum = sb.tile([P, GCOLS], F16)
            dscr = sb.tile([P, GCOLS], F16)
            rdeng = (nc.sync, nc.scalar, nc.gpsimd)
            for i in range(3):
                a, b = i * TH, (i + 1) * TH
                rdeng[i].dma_start(gsum[:, a:b], cout[:, a:b])
            for i in range(3):
                a, b = i * TH, (i + 1) * TH
                nc.vector.scalar_tensor_tensor(
                    dscr[:, a:b], gxp[:, a:b], 1.0, gsum[:, a:b],
                    mybir.AluOpType.mult, mybir.AluOpType.mult,
                    accum_out=stage[:, i:i + 1],
                )

            # ---- partition reduction via PE (ones^T @ stage), reusing a
            # freed slab bank; then fold to the 2-scalar AR2 payload ----
            pfin = slabs[5][0:1, 0:5]
            nc.tensor.matmul(pfin, ones[:, 0:1], stage[:, 0:5],
                             start=True, stop=True)
            ffin = sb.tile([1, 5], F32)
            nc.vector.tensor_copy(ffin[:], pfin)
            fres = sb.tile([1, 4], F32)
            # fres0 = d0+d1; fres1 = fres0+d2; fres2 = fres1*inv/SCALE
            nc.vector.scalar_tensor_tensor(
                fres[:, 0:1], ffin[:, 0:1], 1.0, ffin[:, 1:2],
                mybir.AluOpType.mult, mybir.AluOpType.add,
            )
            nc.vector.scalar_tensor_tensor(
                fres[:, 1:2], fres[:, 0:1], 1.0, ffin[:, 2:3],
                mybir.AluOpType.mult, mybir.AluOpType.add,
            )
            nc.vector.tensor_scalar_mul(fres[:, 2:3], fres[:, 1:2],
                                        inv_nn1 / SCALE)
            # fres3 = zlin_scaled + zsq_scaled  (both subtracted at the end)
            nc.vector.scalar_tensor_tensor(
                fres[:, 3:4], ffin[:, 3:4], 1.0, ffin[:, 4:5],
                mybir.AluOpType.mult, mybir.AluOpType.add,
            )

            cin2 = dram.tile([1, 2], F32)
            cout2 = dram.tile([1, 2], F32, addr_space="Shared")
            nc.gpsimd.dma_start(cin2[:], fres[0:1, 2:4])
            nc.gpsimd.collective_compute(
                "AllReduce",
                mybir.AluOpType.add,
                replica_groups=[list(range(N_CORES))],
                ins=[cin2.opt()],
                outs=[cout2.opt()],
            )

            rb2 = sb.tile([1, 2], F32)
            nc.sync.dma_start(rb2[:], cout2[:])
            res = sb.tile([1, 1], F32)
            nc.vector.tensor_sub(res[:], rb2[0:1, 0:1], rb2[0:1, 1:2])
            nc.sync.dma_start(loss_ap[:], res[0:1, 0:1])

    nc.compile()
    return nc


def _get_nc():
    if "nc" not in _CACHE:
        _CACHE["nc"] = _build()
    return _CACHE["nc"]


def _run(x, y, trace=False, **trace_kwargs):
    nc = _get_nc()
    x = np.ascontiguousarray(np.asarray(x, dtype=np.float32))
    y = np.ascontiguousarray(np.asarray(y, dtype=np.float32))
    assert x.shape == (N, D) and y.shape == (N, D)
    in_maps = [
        {"x": x[c * ROWS:(c + 1) * ROWS], "y": y[c * ROWS:(c + 1) * ROWS]}
        for c in range(N_CORES)
    ]
    res = run_bass_kernel_spmd(nc, in_maps, list(range(N_CORES)), trace=trace,
                               **trace_kwargs)
    loss = np.float32(res.results[0]["loss"][0, 0])
    return np.asarray(loss, dtype=np.float32).reshape(()), res


def kernel(x, y):
    out, _ = _run(x, y, trace=False)
    return out


# revision 6
# speedup vs baseline: 1.1262x; 1.1262x over previous
"""L2 (spectral) contrastive loss on 8 Trainium2 NeuronCores.

Math: with G_x = x.T @ x and G_y = y.T @ y (both [D, D]),
    sum_{i,j} <x_i, y_j>^2 = ||x @ y.T||_F^2 = tr(G_x @ G_y) = sum(G_x * G_y)
so the loss needs only the two Gram matrices (2*N*D^2 MACs) instead of the
[N, N] pairwise product (N^2*D MACs).

v2 split-collective structure (vs the single 1.38MB AllReduce baseline):
  - Rows are split across the 8 cores. Only G_y's partials cross the wire:
      sum(Gx_tot * Gy_tot) = sum_c sum(Gx_c * Gy_tot)
    so AllReduce #1 carries just the G_y triangle (fp16, 688KB), each core
    dots its LOCAL Gx partial against the summed Gy, and AllReduce #2
    carries only [1, 2] f32 scalars.
  - Inputs stream on 4 DMA queues (y chunks first), fp8 casts on the
    scalar engine (gpsimd casts were 6x slower and the old critical path),
    G_y computed kk-outer with all 6 triangle slabs resident in PSUM
    (exactly 8 banks) so the Gram finishes with the last y cast and the
    first collective triggers ~30µs earlier than baseline.
  - z_i = <x_i, y_i> on the vector engine from fp32; z terms ride
    AllReduce #2 in exact f32.
  (A 3-round remote-DMA butterfly is faster than the collective but
  intermittently wedges the device - keep the firmware collectives.)
"""
import numpy as np
from contextlib import ExitStack

from concourse import bacc, tile, mybir
from concourse.bass_utils import run_bass_kernel_spmd

N_CORES = 8
N, D = 8192, 768
ROWS = N // N_CORES          # 1024 rows per core
P = 128                      # SBUF partitions
KCH = ROWS // P              # 8 contraction chunks per core
KK = KCH // 2                # 4 DoubleRow steps (2 chunks per pass)
MS = D // P                  # 6 output slabs per Gram

WIDTHS = [D - P * m for m in range(MS)]              # [768,640,512,384,256,128]
COFF = [sum(WIDTHS[:m]) for m in range(MS)]          # prefix offsets
GCOLS = sum(WIDTHS)                                  # 2688 triangle cols

F32 = mybir.dt.float32
F16 = mybir.dt.float16
FP8 = mybir.dt.float8e4

# Gx pre-scale: keeps fp16 dot products (diag ~8192 * 8192) in fp16 range
SCALE = 2.0 ** -13

_CACHE = {}


def _mm_chunks(width):
    """Split [0, width) at the 512-column PSUM bank boundary."""
    if width <= 512:
        return [(0, width)]
    return [(0, 512), (512, width)]


def _build():
    nc = bacc.Bacc("TRN2", target_bir_lowering=False, debug=False,
                   num_devices=N_CORES)
    x_ap = nc.dram_tensor("x", [ROWS, D], F32, kind="ExternalInput").ap()
    y_ap = nc.dram_tensor("y", [ROWS, D], F32, kind="ExternalInput").ap()
    loss_ap = nc.dram_tensor("loss", [1, 1], F32, kind="ExternalOutput").ap()

    inv_nn1 = 1.0 / (float(N) * (N - 1))

    with tile.TileContext(nc) as tc:
        with ExitStack() as ctx:
            sb = ctx.enter_context(tc.tile_pool(name="sb", bufs=1))
            ps = ctx.enter_context(tc.tile_pool(name="ps", bufs=1, space="PSUM"))
            dram = ctx.enter_context(tc.tile_pool(name="dram", bufs=1,
                                                  space="DRAM"))

            # ---- load inputs: [1024, 768] -> [128p, 8k, 768] on 4 DMA
            # queues, ALL y chunks issued before any x so G_y (the only
            # Gram that rides the big collective) completes first ----
            xt = sb.tile([P, KCH, D], F32)
            yt = sb.tile([P, KCH, D], F32)
            xr = x_ap.rearrange("(n p) d -> p n d", p=P)
            yr = y_ap.rearrange("(n p) d -> p n d", p=P)
            qeng = (nc.sync, nc.scalar, nc.gpsimd)
            for k in range(KCH):
                qeng[k % 3].dma_start(yt[:, k, :], yr[:, k, :])
            for k in range(KCH):
                qeng[k % 3].dma_start(xt[:, k, :], xr[:, k, :])

            # ---- fp8 casts, all on the scalar engine (~0.7µs each; the
            # old gpsimd path was 3.2µs each and the critical path) ----
            yb = sb.tile([P, KCH, D], FP8)
            xb = sb.tile([P, KCH, D], FP8)
            for k in range(KCH):
                nc.scalar.copy(yb[:, k, :], yt[:, k, :])
            for k in range(KCH):
                nc.scalar.copy(xb[:, k, :], xt[:, k, :])

            # partition-reduce weights: ones scaled by inv_nn1/SCALE so the
            # PE reduce also applies the dot normalization; the z columns
            # are pre-scaled to compensate (see stage cols 3-4)
            C0 = inv_nn1 / SCALE
            wvec = sb.tile([P, 1], F32)
            nc.vector.memset(wvec[:], C0)

            # ---- G_y: kk-outer with all 6 triangle slabs resident in
            # PSUM (2+2+1+1+1+1 = exactly 8 banks), consuming each cast
            # chunk pair as it arrives ----
            slabs = []
            for m in range(MS):
                slab = ps.tile([P, WIDTHS[m]], F32, tag=f"slab{m}", bufs=1,
                               name=f"slab{m}")
                slabs.append(slab)

            pack = sb.tile([P, GCOLS], F16)       # G_y fp16 AR payload
            gxp = sb.tile([P, GCOLS], F16)        # local Gx, scaled, x2 off-diag

            def gram_pass(src):
                for kk in range(KK):
                    for m in range(MS):
                        for (c0, c1) in _mm_chunks(WIDTHS[m]):
                            nc.tensor.matmul(
                                slabs[m][:, c0:c1],
                                src[:, 2 * kk:2 * kk + 2, P * m:P * (m + 1)],
                                src[:, 2 * kk:2 * kk + 2,
                                    P * m + c0:P * m + c1],
                                start=(kk == 0),
                                stop=(kk == KK - 1),
                                perf_mode=mybir.MatmulPerfMode.DoubleRow,
                                skip_group_check=True,
                            )

            gram_pass(yb)

            # ---- pack G_y slabs to fp16 (vector) and stage to DRAM in 3
            # pieces (gpsimd queue) so the collective triggers as soon as
            # the last slab lands ----
            cin = dram.tile([P, GCOLS], F16)
            cout = dram.tile([P, GCOLS], F16, addr_space="Shared")
            for m in range(MS):
                off, w = COFF[m], WIDTHS[m]
                nc.vector.tensor_copy(pack[:, off:off + w], slabs[m][:, 0:w])
                if m % 2 == 1:  # stage after slabs {0,1}, {2,3}, {4,5} pack
                    a = COFF[m - 1]
                    b = off + w
                    nc.gpsimd.dma_start(cin[:, a:b], pack[:, a:b])

            nc.gpsimd.collective_compute(
                "AllReduce",
                mybir.AluOpType.add,
                replica_groups=[list(range(N_CORES))],
                ins=[cin.opt()],
                outs=[cout.opt()],
            )

            # ---- G_x into the same PSUM slabs (WAR on the pack copies);
            # packed locally with the 2^-13 pre-scale and the x2 weight on
            # strict-right (off-diagonal) columns; never leaves the core ----
            gram_pass(xb)
            for m in range(MS):
                off, w = COFF[m], WIDTHS[m]
                nc.scalar.mul(gxp[:, off:off + P], slabs[m][:, 0:P], SCALE)
                if w > P:
                    nc.scalar.mul(gxp[:, off + P:off + w],
                                  slabs[m][:, P:w], 2.0 * SCALE)

            # ---- diagonal terms z_i = <x_i, y_i> from fp32 (vector) ----
            zscr = sb.tile([P, D], F32)
            zcols = sb.tile([P, KCH], F32)
            for k in range(KCH):
                nc.vector.scalar_tensor_tensor(
                    zscr[:], xt[:, k, :], 1.0, yt[:, k, :],
                    mybir.AluOpType.mult, mybir.AluOpType.mult,
                    accum_out=zcols[:, k:k + 1],
                )
            zsq = sb.tile([P, KCH], F32)
            stage = sb.tile([P, 5], F32)
            # stage cols 3-4 carry the z terms pre-divided by C0 (and
            # negated, since they're subtracted) so that C0 * sum_p(stage)
            # via the PE reduce yields the finished per-core contribution:
            #   col 3 -> -(2/N)*sum z,  col 4 -> -inv_nn1*sum z^2
            zred = sb.tile([P, 2], F32)
            nc.vector.tensor_reduce(zred[:, 0:1], zcols[:],
                                    mybir.AxisListType.X, mybir.AluOpType.add)
            nc.vector.scalar_tensor_tensor(
                zsq[:], zcols[:], 1.0, zcols[:],
                mybir.AluOpType.mult, mybir.AluOpType.mult,
                accum_out=zred[:, 1:2],
            )
            nc.vector.tensor_scalar_mul(stage[:, 3:4], zred[:, 0:1],
                                        -2.0 * SCALE * (N - 1))
            nc.vector.tensor_scalar_mul(stage[:, 4:5], zred[:, 1:2], -SCALE)

            # ---- after AR1: read back Gy_tot in 3 pieces on 3 queues,
            # dot against the local Gx pack (3 fp16 STTs, f32 col accums) ----
            TH = GCOLS // 3
            gsum = sb.tile([P, GCOLS], F16)
            dscr = sb.tile([P, GCOLS], F16)
            rdeng = (nc.sync, nc.scalar, nc.gpsimd)
            for i in range(3):
                a, b = i * TH, (i + 1) * TH
                rdeng[i].dma_start(gsum[:, a:b], cout[:, a:b])
            for i in range(3):
                a, b = i * TH, (i + 1) * TH
                nc.vector.scalar_tensor_tensor(
                    dscr[:, a:b], gxp[:, a:b], 1.0, gsum[:, a:b],
                    mybir.AluOpType.mult, mybir.AluOpType.mult,
                    accum_out=stage[:, i:i + 1],
                )

            # ---- partition reduction via PE (wvec^T @ stage) applies the
            # C0 normalization; a single horizontal reduce then yields the
            # finished per-core loss contribution, so the scalar AllReduce
            # output IS the loss ----
            pfin = slabs[5][0:1, 0:5]
            nc.tensor.matmul(pfin, wvec[:, 0:1], stage[:, 0:5],
                             start=True, stop=True)
            ffin = sb.tile([1, 5], F32)
            nc.vector.tensor_copy(ffin[:], pfin)
            lc = sb.tile([1, 1], F32)
            nc.vector.tensor_reduce(lc[:], ffin[:],
                                    mybir.AxisListType.X, mybir.AluOpType.add)

            cin2 = dram.tile([1, 1], F32)
            cout2 = dram.tile([1, 1], F32, addr_space="Shared")
            nc.gpsimd.dma_start(cin2[:], lc[:])
            nc.gpsimd.collective_compute(
                "AllReduce",
                mybir.AluOpType.add,
                replica_groups=[list(range(N_CORES))],
                ins=[cin2.opt()],
                outs=[cout2.opt()],
            )
            nc.sync.dma_start(loss_ap[:], cout2[:])

    nc.compile()
    return nc


def _get_nc():
    if "nc" not in _CACHE:
        _CACHE["nc"] = _build()
    return _CACHE["nc"]


def _run(x, y, trace=False, **trace_kwargs):
    nc = _get_nc()
    x = np.ascontiguousarray(np.asarray(x, dtype=np.float32))
    y = np.ascontiguousarray(np.asarray(y, dtype=np.float32))
    assert x.shape == (N, D) and y.shape == (N, D)
    in_maps = [
        {"x": x[c * ROWS:(c + 1) * ROWS], "y": y[c * ROWS:(c + 1) * ROWS]}
        for c in range(N_CORES)
    ]
    res = run_bass_kernel_spmd(nc, in_maps, list(range(N_CORES)), trace=trace,
                               **trace_kwargs)
    loss = np.float32(res.results[0]["loss"][0, 0])
    return np.asarray(loss, dtype=np.float32).reshape(()), res


def kernel(x, y):
    out, _ = _run(x, y, trace=False)
    return out
